# revision 1
# baseline (speedup 1.0000x reference)
"""Trainium2 Bass kernel for BilinearScoringFunction.

scores[b] = relu( einsum('bi,hij,bj->bh', head, W_R, tail)
                  + concat(head, tail) @ V_R.T + b_R ) @ u_R

B=4096, D=512, H=256. Sharded over 8 NeuronCores along the hidden dim H
(32 hidden units per core); each core computes partial u_R dot products
over its hidden slice, and the host sums the 8 partial score vectors.

Per core structure (all matmuls in float32r = TF32 rate, fp32 PSUM accum):
  phase 2 (dominant): per h: T_h = head @ W_h on TensorE (4 accumulating
    K=128 matmuls, N=512), then one fused VectorE custom-DVE
    TENSOR_TENSOR_REDUCE computes bil[:, h] = rowsum(T_h * tail) straight
    out of PSUM.
  linear term (inserted into the PE stream after h=5): h-major
    lin^T = V_slice @ concat^T as 8 accumulating matmuls per 512-batch
    tile with the tiny V chunks stationary, bias added in PSUM, then
    32x32 VectorE block transposes back to b-major.
  phase 3: per batch tile: bil + lin (VectorE), relu (ScalarE),
    fused dot with u_slice (VectorE custom-DVE reduce).

DMA order: W_0 first, then head^T/tail as interleaved per-batch-tile
slices, so the first matmul quad gates on ~1.5MB and h=0/h=1 run
DMA-paced as the streams land.
"""

import os
from contextlib import ExitStack

import numpy as np

import concourse.bacc as bacc
import concourse.tile as tile
import concourse.mybir as mybir
from concourse import bass_utils
from concourse.dve_ops import TENSOR_TENSOR_REDUCE

B, D, H = 4096, 512, 256
NCORES = 8
HSL = H // NCORES          # hidden units per core = 32
P = 128                    # partitions
BT = B // P                # batch tiles of 128 = 32
NB5 = B // 512             # batch tiles of 512 = 8
KD = D // P                # contraction chunks per operand = 4
KC = 2 * KD                # concat contraction chunks = 8
LIN_AT_H = 6               # insert linear-term matmuls before this h

_F32 = mybir.dt.float32
_F32R = mybir.dt.float32r

_NC_CACHE = None


def _build_nc():
    nc = bacc.Bacc(
        "TRN2",
        target_bir_lowering=False,
        debug=False,
        enable_asserts=False,
        num_devices=NCORES,
    )
    hT = nc.dram_tensor("hT", [D, B], _F32R, kind="ExternalInput").ap()
    tT = nc.dram_tensor("tT", [D, B], _F32R, kind="ExternalInput").ap()
    tl = nc.dram_tensor("tl", [B, D], _F32, kind="ExternalInput").ap()
    w = nc.dram_tensor("w", [HSL, D, D], _F32R, kind="ExternalInput").ap()
    vc = nc.dram_tensor("vc", [2 * D, HSL], _F32R, kind="ExternalInput").ap()
    ub = nc.dram_tensor("ub", [P, HSL], _F32, kind="ExternalInput").ap()
    br = nc.dram_tensor("br", [HSL, 1], _F32, kind="ExternalInput").ap()
    out = nc.dram_tensor("scores_part", [P, BT], _F32, kind="ExternalOutput").ap()

    with tile.TileContext(nc) as tc, ExitStack() as ctx:
        const = ctx.enter_context(tc.tile_pool(name="const", bufs=1))
        wp = ctx.enter_context(tc.tile_pool(name="w", bufs=3))
        psp = ctx.enter_context(tc.tile_pool(name="ps", bufs=6, space="PSUM"))
        lps = ctx.enter_context(tc.tile_pool(name="lps", bufs=2, space="PSUM"))
        scr = ctx.enter_context(tc.tile_pool(name="scr", bufs=2))
        tTp = ctx.enter_context(tc.tile_pool(name="tT", bufs=1))

        # --- DMAs in priority order: compute start gates on W[0] + hT only.
        w_tiles = {}

        def load_w(h):
            w_t = wp.tile([P, KD, D], _F32R, name="wt")
            nc.sync.dma_start(w_t[:], w[h].rearrange("(k p) j -> p k j", p=P))
            return w_t

        w_tiles[0] = load_w(0)
        w_tiles[1] = load_w(1)

        # head^T and tail as interleaved per-batch-tile slices: the h=0
        # matmul quad for tile bt only gates on hT[bt] (256KB), so compute
        # starts ~6us in and h=0 runs DMA-paced instead of idling ~38us.
        hT_t = const.tile([P, KD, B], _F32R)
        tl_t = const.tile([P, BT, D], _F32)
        hT_r = hT.rearrange("(k p) b -> p k b", p=P)
        for bt in range(BT):
            nc.sync.dma_start(
                hT_t[:, :, bt * P:(bt + 1) * P], hT_r[:, :, bt * P:(bt + 1) * P]
            )
            nc.sync.dma_start(
                tl_t[:, bt, :], tl[bt * P:(bt + 1) * P, :]
            )

        vc_t = const.tile([P, KC, HSL], _F32R)
        nc.sync.dma_start(vc_t[:], vc.rearrange("(k p) h -> p k h", p=P))
        ub_t = const.tile([P, HSL], _F32)
        nc.sync.dma_start(ub_t[:], ub[:, :])
        br_t = const.tile([HSL, 1], _F32)
        nc.sync.dma_start(br_t[:], br[:, :])

        # tail^T chunks for the linear term: [128 rows of D, 2048 cols of B]
        # tag per k-chunk, two halves of B cycle through each tag's slot.
        def load_tT(k, half):
            t = tTp.tile([P, B // 2], _F32R, name=f"tTk{k}")
            nc.sync.dma_start(
                t[:], tT[k * P:(k + 1) * P, half * (B // 2):(half + 1) * (B // 2)]
            )
            return t

        bil_t = const.tile([P, BT, HSL], _F32)   # pure bilinear, b-major
        linb_t = const.tile([P, BT, HSL], _F32)  # linear + bias, b-major
        scores_t = const.tile([P, BT], _F32)

        lsp = ctx.enter_context(tc.tile_pool(name="lst", bufs=2))

        def lin_phase():
            for half in range(2):
                tT_tiles = [load_tT(k, half) for k in range(KD)]
                for b5 in range(NB5 // 2):
                    b512 = half * (NB5 // 2) + b5
                    pl = lps.tile([HSL, 512], _F32, name="pl")
                    for kc in range(KC):
                        if kc < KD:
                            rhs = hT_t[:, kc, b512 * 512:(b512 + 1) * 512]
                        else:
                            rhs = tT_tiles[kc - KD][:, b5 * 512:(b5 + 1) * 512]
                        nc.tensor.matmul(
                            pl[:], vc_t[:, kc, :], rhs,
                            start=(kc == 0), stop=(kc == KC - 1),
                        )
                    # bias add in place (per-partition scalar = b_R slice)
                    nc.vector.tensor_scalar_add(pl[:], pl[:], br_t[:])
                    # transpose all 16 32x32 blocks in one DVE op
                    lin_stage = lsp.tile([HSL, 512], _F32, name="lst")
                    nc.vector.transpose(lin_stage[:], pl[:])
                    # scatter blocks to b-major linb_t via 4 SBUF->SBUF DMAs:
                    # block j holds b-rows for partitions (j%4)*32.., bt b512*4+j//4
                    for m in range(4):
                        src = lin_stage.rearrange(
                            "p (t m c) -> p t m c", t=4, m=4
                        )[:, :, m, :]  # [32, 4, 32]: t-th block with j%4==m
                        dst = linb_t[m * 32:(m + 1) * 32,
                                     b512 * 4:(b512 + 1) * 4, :]
                        nc.sync.dma_start(dst, src)

        # --- Phase 2: per h: T_h = head @ W_h ; bil[:, h] = rowsum(T_h * tail)
        # On the last h, phase-3 relu prep is interleaved per batch tile.
        s2p = ctx.enter_context(tc.tile_pool(name="s2", bufs=2))

        def _udot(bt):
            # scores_part[b] = relu(bil + lin)[b, :] @ u_slice
            s2_t = s2p.tile([P, HSL], _F32, name="s2")
            nc.vector._custom_dve(
                TENSOR_TENSOR_REDUCE,
                out=s2_t[:],
                in0=bil_t[:, bt, :],
                in1=ub_t[:],
                s0=0.0,
                s1=1.0,
                accum_out=scores_t[:, bt:bt + 1],
            )
        def quad(h, bt, w_t):
            ps_t = psp.tile([P, D], _F32, name="ps")
            for k in range(KD):
                nc.tensor.matmul(
                    ps_t[:],
                    hT_t[:, k, bt * P:(bt + 1) * P],
                    w_t[:, k, :],
                    start=(k == 0),
                    stop=(k == KD - 1),
                )
            s_t = scr.tile([P, D], _F32, name="s")
            nc.vector._custom_dve(
                TENSOR_TENSOR_REDUCE,
                out=s_t[:],
                in0=ps_t[:],
                in1=tl_t[:, bt, :],
                s0=0.0,
                s1=1.0,
                accum_out=bil_t[:, bt, h:h + 1],
            )

        # h=0 and h=1 interleaved per bt: during this window the hT/tl
        # streams are still landing, so give the PE 2 quads per arriving tile.
        for bt in range(BT):
            quad(0, bt, w_tiles[0])
            quad(1, bt, w_tiles[1])
        w_tiles.pop(0)
        w_tiles.pop(1)
        w_tiles[2] = load_w(2)

        for h in range(2, HSL):
            if h + 1 < HSL and (h + 1) not in w_tiles:
                w_tiles[h + 1] = load_w(h + 1)
            w_t = w_tiles.pop(h)
            for bt in range(BT):
                quad(h, bt, w_t)
                if h == HSL - 1:
                    # in-place: bil := relu(bil + lin)
                    nc.vector.tensor_add(
                        bil_t[:, bt, :], bil_t[:, bt, :], linb_t[:, bt, :]
                    )
                    nc.scalar.activation(
                        bil_t[:, bt, :], bil_t[:, bt, :],
                        mybir.ActivationFunctionType.Relu,
                    )
                    if bt >= 1:
                        _udot(bt - 1)
                    if bt - 1 == 15:
                        nc.sync.dma_start(out[:, 0:16], scores_t[:, 0:16])
            if h == LIN_AT_H - 1:
                lin_phase()

        _udot(BT - 1)
        nc.sync.dma_start(out[:, 16:BT], scores_t[:, 16:BT])

    nc.compile()
    return nc


def _get_nc():
    global _NC_CACHE
    if _NC_CACHE is None:
        _NC_CACHE = _build_nc()
    return _NC_CACHE


def kernel(head_embeddings, relation_embeddings, tail_embeddings, W_R, V_R, u_R, b_R):
    head = np.asarray(head_embeddings, dtype=np.float32)
    tail = np.asarray(tail_embeddings, dtype=np.float32)
    W = np.asarray(W_R, dtype=np.float32)
    V = np.asarray(V_R, dtype=np.float32)
    u = np.asarray(u_R, dtype=np.float32)
    b = np.asarray(b_R, dtype=np.float32)

    headT = np.ascontiguousarray(head.T)
    tailT = np.ascontiguousarray(tail.T)

    in_maps = []
    for c in range(NCORES):
        hs = slice(c * HSL, (c + 1) * HSL)
        in_maps.append({
            "hT": headT,
            "tT": tailT,
            "tl": tail,
            "w": np.ascontiguousarray(W[hs]),
            "vc": np.ascontiguousarray(V[hs].T),
            "ub": np.ascontiguousarray(np.broadcast_to(u[hs], (P, HSL))),
            "br": np.ascontiguousarray(b[hs].reshape(HSL, 1)),
        })

    nc = _get_nc()
    trace = bool(int(os.environ.get("BILINEAR_TRACE", "0")))
    res = bass_utils.run_bass_kernel_spmd(
        nc, in_maps, core_ids=list(range(NCORES)), trace=trace
    )
    if trace:
        print(f"HW exec time: {res.exec_time_ns} ns")
        if res.instructions_and_trace:
            print(f"trace: {res.instructions_and_trace[1]}")

    acc = np.zeros(B, dtype=np.float64)
    for c in range(NCORES):
        part = res.results[c]["scores_part"]  # [P, BT]
        acc += part.T.reshape(-1).astype(np.float64)
    return acc.astype(np.float32)

